# revision 16
# baseline (speedup 1.0000x reference)
"""Self-contained Trainium2 Bass kernel for a 2D-RoPE multi-head attention block.

Full module: qkv = x @ w_qkv.T + b_qkv; 2D rope on q,k; softmax attention;
out proj.  B=2, N=2048, C=768, H=12, Dh=64.

Sharding: 8 cores = 2 batches x 4 query-chunks of 512 tokens.  Each core
computes K,V for its batch's full 2048 tokens (replicated within the
4-core group) and Q for its own 512-token chunk; attention and the output
projection are then fully local.  Host only slices/transposes inputs and
concatenates outputs.
"""

import sys

import numpy as np

for _p in ("/opt/trn_rl_repo", "/root/.axon_site/_ro/trn_rl_repo"):
    if _p not in sys.path:
        sys.path.append(_p)

import concourse.bass as bass
import concourse.tile as tile
from concourse import mybir
from concourse.vector_clock import ScopedClock
import bass_rust

F32 = mybir.dt.float32
F32R = mybir.dt.float32r
BF16 = mybir.dt.bfloat16
AF = mybir.ActivationFunctionType

B, N_FULL, C, H, DH = 2, 2048, 768, 12, 64
NQ_FULL = N_FULL // 4  # 512 query rows per core
CT = C // 128  # contraction tiles
ROPE_FREQ = 100.0


class _TC(tile.TileContext):
    """TileContext whose tail drain splits sem waits one-per-instruction.

    The walrus in this container rejects >1 sync wait on a CTRL-class
    instruction ("Too many sync wait commands"), and the stock tail drain
    carries one wait per active processor.
    """

    def _drain_and_barrier(self, tick_clock, wait_clock):
        nc = self.nc
        drain_inst = nc.sync.drain()
        wait_clock.add_sem_waits(
            drain_inst.ins, ScopedClock({None: tick_clock.global_clock})
        )
        si = drain_inst.ins.sync_info
        waits = list(si.on_wait) if si is not None else []
        if len(waits) > 1:
            drain_inst.ins.sync_info = bass_rust.SyncInfo(
                on_wait=waits[:1], on_update=si.on_update
            )
            for w in waits[1:]:
                n = nc.sync.nop(nofuse=True)
                n.ins.sync_info = bass_rust.SyncInfo(on_wait=[w], on_update=[])
        nc.all_engine_barrier()
        popped = nc._tile_sem_poison_stack.pop()
        assert popped is self._sem_poison
        nc.clear_and_free_semaphores(list(self.sems.allocated().values()))
        nc.all_engine_barrier()


def _split_multi_waits(nc):
    """This container's walrus accepts at most ONE sync wait per instruction.

    Move extra waits onto fresh NOPs inserted just before the instruction
    (same engine, same basic block) - semantically identical stalls.
    """
    for f in nc.m.functions:
        for bb in f.blocks:
            il = bb.instructions
            if not any(
                inst.sync_info is not None and len(inst.sync_info.on_wait) > 1
                for inst in il
            ):
                continue
            new = []
            for inst in il:
                si = inst.sync_info
                if si is not None and len(si.on_wait) > 1:
                    waits = list(si.on_wait)
                    for w in waits[:-1]:
                        nop = mybir.InstNoOp(
                            name=nc.get_next_instruction_name(), ins=[], outs=[]
                        )
                        nop.engine = inst.engine
                        nop.sync_info = bass_rust.SyncInfo(
                            on_wait=[w], on_update=[]
                        )
                        nc.register_instruction(nop)
                        new.append(nop)
                    inst.sync_info = bass_rust.SyncInfo(
                        on_wait=[waits[-1]], on_update=si.on_update
                    )
                new.append(inst)
            il[:] = new


def build_kernel(n=N_FULL, nq=NQ_FULL, repeat=1):
    """Build the per-core SPMD program.  n = kv tokens, nq = query rows.

    repeat>1 re-emits the whole body N times (timing calibration only).
    """
    assert n % 512 == 0 or n in (128, 256, 512)
    jt = n // 128       # key tiles
    jg2 = jt // 2       # key-tile pairs (psum/exp groups)
    vtt = n // 128      # v token tiles
    hp_n = H // 2       # head pairs
    tch = max(1, n // 512)   # 512-col chunks of n
    tcw = min(512, n)
    kch_n = max(1, n // 1024)  # 1024-col evac chunks
    kchw = min(1024, n)

    nc = bass.Bass("TRN2")
    xT = nc.dram_tensor("xT", [C, n], F32, kind="ExternalInput")
    xqT = nc.dram_tensor("xqT", [C, nq], F32, kind="ExternalInput")
    cosT = nc.dram_tensor("cosT", [128, n], F32, kind="ExternalInput")
    sinT = nc.dram_tensor("sinT", [128, n], F32, kind="ExternalInput")
    cosq = nc.dram_tensor("cosq", [128, nq], F32, kind="ExternalInput")
    sinq = nc.dram_tensor("sinq", [128, nq], F32, kind="ExternalInput")
    wkqT = nc.dram_tensor("wkqT", [C, 2 * C], F32, kind="ExternalInput")
    wvT = nc.dram_tensor("wvT", [C, C], F32, kind="ExternalInput")
    wpT = nc.dram_tensor("wpT", [C, C], F32, kind="ExternalInput")
    bkq = nc.dram_tensor("bkq", [128, 2 * CT], F32, kind="ExternalInput")
    bv = nc.dram_tensor("bv", [1, C], F32, kind="ExternalInput")
    bp = nc.dram_tensor("bp", [1, C], F32, kind="ExternalInput")
    ones = nc.dram_tensor("ones", [1, 512], F32, kind="ExternalInput")
    r2t = nc.dram_tensor("r2t", [128, 128], F32, kind="ExternalInput")
    y = nc.dram_tensor("y", [nq, C], F32, kind="ExternalOutput")

    with _TC(nc) as tc:
      for _rep in range(repeat):
        with (
            tc.tile_pool(name="const", bufs=1) as pc,
            tc.tile_pool(name="kq", bufs=1) as pkq,
            tc.tile_pool(name="oT", bufs=1) as po,
            tc.tile_pool(name="v", bufs=1) as pv,
            tc.tile_pool(name="x", bufs=1) as px,
        ):
            # ---- constants ----
            bkq_sb = pc.tile([128, 2 * CT], F32, tag="bkq", name="bkq")
            nc.sync.dma_start(bkq_sb[:], bkq[:])
            bv_sb = pc.tile([1, C], F32, tag="bv", name="bv")
            nc.sync.dma_start(bv_sb[:], bv[:])
            bp_sb = pc.tile([1, C], F32, tag="bp", name="bp")
            nc.sync.dma_start(bp_sb[:], bp[:])
            ones_sb = pc.tile([1, 512], F32, tag="ones", name="ones")
            nc.sync.dma_start(ones_sb[:], ones[:])
            r2t_sb = pc.tile([128, 128], F32, tag="r2t", name="r2t")
            nc.sync.dma_start(r2t_sb[:], r2t[:])
            ones64_sb = pc.tile([65, 64], F32, tag="ones64", name="ones64")
            nc.vector.memset(ones64_sb[64:65, :], 1.0)
            cos_sb = pc.tile([128, n], F32, tag="cos", name="cos")
            nc.sync.dma_start(cos_sb[:], cosT[:])
            sin_sb = pc.tile([128, n], F32, tag="sin", name="sin")
            nc.sync.dma_start(sin_sb[:], sinT[:])
            cq_sb = pc.tile([128, nq], F32, tag="cosq", name="cosq")
            nc.sync.dma_start(cq_sb[:], cosq[:])
            sq_sb = pc.tile([128, nq], F32, tag="sinq", name="sinq")
            nc.sync.dma_start(sq_sb[:], sinq[:])

            # ---- resident tensors ----
            kT = []  # roped K^T, bf16, head-pair-major: tile p rows = heads 2p,2p+1
            for p in range(hp_n):
                kT.append(pkq.tile([128, n], BF16, tag=f"kT{p}", name=f"kT{p}"))
            qT = []
            for p in range(hp_n):
                qT.append(pkq.tile([128, nq], BF16, tag=f"qT{p}", name=f"qT{p}"))
            oT = []  # O^T feature-major (post-normalize), f32
            for p in range(hp_n):
                oT.append(po.tile([128, nq], F32, tag=f"oT{p}", name=f"oT{p}"))
            v_sb = []  # token-major V with ones col per head: [128, H*65] bf16
            for t in range(vtt):
                v_sb.append(pv.tile([128, H * 65], BF16, tag=f"v{t}", name=f"v{t}"))
            x_sb = []
            for c in range(CT):
                x_sb.append(px.tile([128, n], F32, tag=f"x{c}", name=f"x{c}"))
                nc.sync.dma_start(x_sb[c][:], xT[c * 128 : (c + 1) * 128, :])
            xq_sb = []
            for c in range(CT):
                xq_sb.append(px.tile([128, nq], F32, tag=f"xq{c}", name=f"xq{c}"))
                nc.sync.dma_start(xq_sb[c][:], xqT[c * 128 : (c + 1) * 128, :])

            # ======== phase 1a: K^T and Q^T (feature-major) + rope ========
            with (
                tc.tile_pool(name="w1", bufs=1) as pw1,
                tc.tile_pool(name="tmp", bufs=2) as ptmp,
                tc.tile_pool(name="psK", bufs=2, space="PSUM") as psK,
                tc.tile_pool(name="psR", bufs=2, space="PSUM") as psR,
            ):
                wkq_sb = []
                for c in range(CT):
                    wkq_sb.append(pw1.tile([128, 2 * C], F32, tag=f"wkq{c}", name=f"wkq{c}"))
                    nc.sync.dma_start(
                        wkq_sb[c][:], wkqT[c * 128 : (c + 1) * 128, :]
                    )

                def rope_block(dst, ps_pool, w_off, o, bias_col, src, src_w,
                               cos_s, sin_s):
                    # dst[o][:, :] (bf16) = rope(W[:, w_off+o*128] ^T @ src + b)
                    nchunks = max(1, src_w // 1024)
                    chw = min(1024, src_w)
                    for ch in range(nchunks):
                        ps = ps_pool.tile([128, chw], F32, tag="k", name="k")
                        nmm = max(1, chw // 512)
                        mmw = min(512, chw)
                        for nn_ in range(nmm):
                            col0 = ch * chw + nn_ * mmw
                            for c in range(CT):
                                nc.tensor.matmul(
                                    ps[:, nn_ * mmw : (nn_ + 1) * mmw],
                                    wkq_sb[c][:, w_off + o * 128 : w_off + (o + 1) * 128].bitcast(F32R),
                                    src[c][:, col0 : col0 + mmw].bitcast(F32R),
                                    start=(c == 0),
                                    stop=(c == CT - 1),
                                )
                        # evac + bias (per-partition scalar add), keep f32
                        tev = ptmp.tile([128, chw], F32, tag="tev", name="tev")
                        nc.vector.tensor_scalar_add(
                            tev[:], ps[:], bkq_sb[:, bias_col : bias_col + 1]
                        )
                        # rotate via PE: psR = R2 @ tev
                        pr = psR.tile([128, chw], F32, tag="r", name="r")
                        for nn_ in range(nmm):
                            nc.tensor.matmul(
                                pr[:, nn_ * mmw : (nn_ + 1) * mmw],
                                r2t_sb[:].bitcast(F32R),
                                tev[:, nn_ * mmw : (nn_ + 1) * mmw].bitcast(F32R),
                                start=True,
                                stop=True,
                            )
                        # combine: dst = tev*cos + (R2 tev)*sin
                        c0 = ch * chw
                        t1 = ptmp.tile([128, chw], F32, tag="t1", name="t1")
                        nc.gpsimd.tensor_mul(
                            t1[:], tev[:], cos_s[:, c0 : c0 + chw]
                        )
                        t2 = ptmp.tile([128, chw], F32, tag="t2", name="t2")
                        nc.vector.tensor_mul(
                            t2[:], pr[:], sin_s[:, c0 : c0 + chw]
                        )
                        nc.vector.tensor_add(
                            dst[:, c0 : c0 + chw], t1[:], t2[:]
                        )

                for o in range(CT):  # K head-pair tiles
                    rope_block(kT[o], psK, 0, o, o, x_sb, n, cos_sb, sin_sb)
                for o in range(CT):  # Q head-pair tiles
                    rope_block(qT[o], psK, C, o, CT + o, xq_sb, nq, cq_sb, sq_sb)

            # ======== phase 1b: V (token-major) ========
            with (
                tc.tile_pool(name="w2", bufs=1) as pw2,
                tc.tile_pool(name="psV", bufs=2, space="PSUM") as psV,
            ):
                wv_sb = []
                for c in range(CT):
                    wv_sb.append(pw2.tile([128, C], F32, tag=f"wv{c}", name=f"wv{c}"))
                    nc.sync.dma_start(wv_sb[c][:], wvT[c * 128 : (c + 1) * 128, :])
                for t in range(vtt):
                    ps = psV.tile([128, C], F32, tag="v", name="v")
                    for e0, ew in ((0, 512), (512, 256)):
                        for c in range(CT):
                            nc.tensor.matmul(
                                ps[:, e0 : e0 + ew],
                                x_sb[c][:, t * 128 : (t + 1) * 128].bitcast(F32R),
                                wv_sb[c][:, e0 : e0 + ew].bitcast(F32R),
                                start=(c == 0),
                                stop=False,
                            )
                        nc.tensor.matmul(  # + ones[t] x bv  (bias)
                            ps[:, e0 : e0 + ew],
                            ones_sb[0:1, 0:128].bitcast(F32R),
                            bv_sb[0:1, e0 : e0 + ew].bitcast(F32R),
                            start=False,
                            stop=True,
                        )
                    vview = v_sb[t][:].rearrange("p (h e) -> p h e", e=65)
                    psview = ps[:].rearrange("p (h d) -> p h d", d=DH)
                    nc.vector.tensor_copy(vview[:, :, 0:DH], psview[:, :, :])
                    nc.vector.memset(vview[:, :, DH : DH + 1], 1.0)

            # ======== phase 2: attention (S^T layout, flash-free) ========
            with (
                tc.tile_pool(name="pT", bufs=2) as ppt,
                tc.tile_pool(name="inv", bufs=2) as pinv,
                tc.tile_pool(name="oscr", bufs=2) as poscr,
                tc.tile_pool(name="psS", bufs=2, space="PSUM") as psS,
                tc.tile_pool(name="psO", bufs=2, space="PSUM") as psO,
            ):
                for hp in range(hp_n):
                    hA, hB = 2 * hp, 2 * hp + 1
                    oA = psO.tile([65, nq], F32, tag="oA", name="oA")
                    oB = psO.tile([65, nq], F32, tag="oB", name="oB")
                    for g in range(jg2):
                        sA = psS.tile([128, 2 * nq], F32, tag="s", name="s")
                        sB = psS.tile([128, 2 * nq], F32, tag="s", name="s")
                        for jj in range(2):
                            j = 2 * g + jj
                            nc.tensor.matmul(
                                sA[:, jj * nq : (jj + 1) * nq],
                                kT[hp][0:64, j * 128 : (j + 1) * 128],
                                qT[hp][0:64, :],
                                start=True, stop=True,
                                tile_position=(0, 0),
                            )
                            nc.tensor.matmul(
                                sB[:, jj * nq : (jj + 1) * nq],
                                kT[hp][64:128, j * 128 : (j + 1) * 128],
                                qT[hp][64:128, :],
                                start=True, stop=True,
                                tile_position=(64, 0),
                            )
                        pA = ppt.tile([128, 2 * nq], BF16, tag="pA", name="pA")
                        nc.scalar.activation(pA[:], sA[:], AF.Exp, scale=0.125)
                        pB = ppt.tile([128, 2 * nq], BF16, tag="pB", name="pB")
                        nc.scalar.activation(pB[:], sB[:], AF.Exp, scale=0.125)
                        for jj in range(2):
                            j = 2 * g + jj
                            nc.tensor.matmul(
                                oA[:, :],
                                v_sb[j][:, hA * 65 : (hA + 1) * 65],
                                pA[:, jj * nq : (jj + 1) * nq],
                                start=(j == 0), stop=(j == jt - 1),
                            )
                            nc.tensor.matmul(
                                oB[:, :],
                                v_sb[j][:, hB * 65 : (hB + 1) * 65],
                                pB[:, jj * nq : (jj + 1) * nq],
                                start=(j == 0), stop=(j == jt - 1),
                            )
                    # normalize: O = O / denom (denom = row 64).
                    # reciprocal at partition 64, broadcast to partitions
                    # 0-63 via a K=1 outer product with a ones row.
                    for which, ops in (("A", oA), ("B", oB)):
                        invr = pinv.tile([65, nq], F32, tag=f"invr{which}",
                                         name=f"invr{which}")
                        with nc.allow_low_precision(reason="feeds f32r matmul"):
                            nc.vector.reciprocal(invr[64:65, :], ops[64:65, :])
                        pib = psS.tile([64, nq], F32, tag="s", name="pib")
                        nc.tensor.matmul(
                            pib[:, :],
                            ones64_sb[64:65, :].bitcast(F32R),
                            invr[64:65, :].bitcast(F32R),
                            start=True, stop=True,
                        )
                        invs = pinv.tile([64, nq], F32, tag=f"invs{which}",
                                         name=f"invs{which}")
                        nc.vector.tensor_copy(invs[:, :], pib[:, :])
                        if which == "A":
                            nc.vector.tensor_mul(
                                oT[hp][0:64, :], ops[0:64, :], invs[:, :]
                            )
                        else:
                            osc = poscr.tile([64, nq], F32, tag="oscr",
                                             name="oscr")
                            nc.vector.tensor_mul(
                                osc[:, :], ops[0:64, :], invs[:, :]
                            )
                            # partition shift 0-63 -> 64-127 via DMA
                            nc.sync.dma_start(oT[hp][64:128, :], osc[:, :])

            # ======== phase 3: output projection ========
            with (
                tc.tile_pool(name="w3", bufs=1) as pw3,
                tc.tile_pool(name="psY", bufs=2, space="PSUM") as psY,
            ):
                wp_sb = []
                for c in range(CT):
                    wp_sb.append(pw3.tile([128, C], F32, tag=f"wp{c}", name=f"wp{c}"))
                    nc.sync.dma_start(wp_sb[c][:], wpT[c * 128 : (c + 1) * 128, :])
                for t in range(nq // 128):
                    ps = psY.tile([128, C], F32, tag="y", name="y")
                    for e0, ew in ((0, 512), (512, 256)):
                        for f in range(CT):
                            nc.tensor.matmul(
                                ps[:, e0 : e0 + ew],
                                oT[f][:, t * 128 : (t + 1) * 128].bitcast(F32R),
                                wp_sb[f][:, e0 : e0 + ew].bitcast(F32R),
                                start=(f == 0),
                                stop=False,
                            )
                        nc.tensor.matmul(  # + ones[t] x bp  (bias)
                            ps[:, e0 : e0 + ew],
                            ones_sb[0:1, 0:128].bitcast(F32R),
                            bp_sb[0:1, e0 : e0 + ew].bitcast(F32R),
                            start=False,
                            stop=True,
                        )
                    ysb = pw3.tile([128, C], F32, tag="ysb", name="ysb", bufs=2)
                    nc.scalar.copy(ysb[:], ps[:])
                    nc.sync.dma_start(y[t * 128 : (t + 1) * 128, :], ysb[:])

    _split_multi_waits(nc)
    return nc


# ---------------------------------------------------------------------------
# host-side prep
# ---------------------------------------------------------------------------

def _rope_tables(xpos_b):
    """cos/sin tables [128, N] (64 dims stacked twice for head-pair tiles)."""
    d = 32
    inv = (
        1.0
        / (np.float32(ROPE_FREQ) ** (np.arange(0, d, 2, dtype=np.float32) / np.float32(d)))
    ).astype(np.float32)
    py = xpos_b[:, 0].astype(np.float32)[:, None] * inv  # [N, 16]
    px = xpos_b[:, 1].astype(np.float32)[:, None] * inv
    emb = np.concatenate([py, py, px, px], axis=1)  # [N, 64]
    cos = np.cos(emb).T.astype(np.float32)  # [64, N]
    sin = np.sin(emb).T.astype(np.float32)
    return (
        np.ascontiguousarray(np.concatenate([cos, cos], axis=0)),
        np.ascontiguousarray(np.concatenate([sin, sin], axis=0)),
    )


def _r2t():
    r32 = np.zeros((32, 32), np.float32)
    r32[0:16, 16:32] = -np.eye(16, dtype=np.float32)
    r32[16:32, 0:16] = np.eye(16, dtype=np.float32)
    r = np.zeros((128, 128), np.float32)
    for i in range(4):
        r[i * 32 : (i + 1) * 32, i * 32 : (i + 1) * 32] = r32
    return np.ascontiguousarray(r.T)


def prep_in_maps(x, xpos, w_qkv, b_qkv, w_proj, b_proj, n=N_FULL, nq=NQ_FULL,
                 n_cores=8):
    x = np.asarray(x, dtype=np.float32)
    xpos = np.asarray(xpos)
    w_qkv = np.asarray(w_qkv, dtype=np.float32)
    b_qkv = np.asarray(b_qkv, dtype=np.float32)
    w_proj = np.asarray(w_proj, dtype=np.float32)
    b_proj = np.asarray(b_proj, dtype=np.float32)

    wkqT = np.ascontiguousarray(
        np.concatenate([w_qkv[C : 2 * C], w_qkv[0:C]], axis=0).T
    )  # [C, 2C]: K cols then Q cols
    wvT = np.ascontiguousarray(w_qkv[2 * C : 3 * C].T)
    wpT = np.ascontiguousarray(w_proj.T)
    bkq = np.ascontiguousarray(
        np.concatenate([b_qkv[C : 2 * C], b_qkv[0:C]]).reshape(2 * CT, 128).T
    )  # [128, 12]
    bv = np.ascontiguousarray(b_qkv[2 * C : 3 * C][None])
    bp = np.ascontiguousarray(b_proj[None])
    ones = np.ones((1, 512), np.float32)
    r2t = _r2t()

    shared = dict(wkqT=wkqT, wvT=wvT, wpT=wpT, bkq=bkq, bv=bv, bp=bp,
                  ones=ones, r2t=r2t)

    per_batch = []
    for b in range(x.shape[0]):
        xT = np.ascontiguousarray(x[b, :n].T)
        cosT, sinT = _rope_tables(np.asarray(xpos[b, :n]))
        per_batch.append((xT, cosT, sinT))

    chunks_per_batch = max(1, n_cores // x.shape[0])
    in_maps = []
    for core in range(n_cores):
        b = core // chunks_per_batch
        q0 = (core % chunks_per_batch) * nq
        xT, cosT, sinT = per_batch[b]
        in_maps.append(
            dict(
                xT=xT,
                xqT=np.ascontiguousarray(xT[:, q0 : q0 + nq]),
                cosT=cosT,
                sinT=sinT,
                cosq=np.ascontiguousarray(cosT[:, q0 : q0 + nq]),
                sinq=np.ascontiguousarray(sinT[:, q0 : q0 + nq]),
                **shared,
            )
        )
    return in_maps


_CACHED = {}


def make_runner(nc, n_cores=8):
    """One cached jitted shard_map callable for the whole session.

    Mirrors bass2jax.run_bass_via_pjrt but hoists jit construction so
    repeat calls skip retracing, and exposes device staging for timing.
    """
    import jax
    from jax.experimental.shard_map import shard_map
    from jax.sharding import Mesh, PartitionSpec

    from concourse import bass2jax, mybir as mb

    bass2jax.install_neuronx_cc_hook()
    assert nc.dbg_addr is None

    partition_name = nc.partition_id_tensor.name if nc.partition_id_tensor else None
    in_names, out_names, out_avals = [], [], []
    for alloc in nc.m.functions[0].allocations:
        if not isinstance(alloc, mb.MemoryLocationSet):
            continue
        name = alloc.memorylocations[0].name
        if alloc.kind == "ExternalInput":
            if name != partition_name:
                in_names.append(name)
        elif alloc.kind == "ExternalOutput":
            out_names.append(name)
            out_avals.append(
                jax.core.ShapedArray(tuple(alloc.tensor_shape), mb.dt.np(alloc.dtype))
            )
    n_params = len(in_names)
    all_in = list(in_names) + list(out_names)
    if partition_name is not None:
        all_in.append(partition_name)
    donate = tuple(range(n_params, n_params + len(out_names)))

    def _body(*args):
        operands = list(args)
        if partition_name is not None:
            operands.append(bass2jax.partition_id_tensor())
        outs = bass2jax._bass_exec_p.bind(
            *operands,
            out_avals=tuple(out_avals),
            in_names=tuple(all_in),
            out_names=tuple(out_names),
            lowering_input_output_aliases=(),
            sim_require_finite=True,
            sim_require_nnan=True,
            nc=nc,
        )
        return tuple(outs)

    devices = jax.devices()[:n_cores]
    mesh = Mesh(np.asarray(devices), ("core",))
    n_outs = len(out_names)
    sharded = jax.jit(
        shard_map(
            _body,
            mesh=mesh,
            in_specs=(PartitionSpec("core"),) * (n_params + n_outs),
            out_specs=(PartitionSpec("core"),) * n_outs,
            check_rep=False,
        ),
        donate_argnums=donate,
        keep_unused=True,
    )

    def stage(in_maps):
        """host in_maps -> concatenated device arrays (inputs only)."""
        concat = [
            jax.device_put(
                np.concatenate([np.asarray(in_maps[c][nm]) for c in range(n_cores)], axis=0)
            )
            for nm in in_names
        ]
        jax.block_until_ready(concat)
        return concat

    def zeros():
        zs = [
            jax.device_put(
                np.zeros((n_cores * av.shape[0], *av.shape[1:]), av.dtype)
            )
            for av in out_avals
        ]
        jax.block_until_ready(zs)
        return zs

    def run(staged_in, staged_zeros):
        outs = sharded(*staged_in, *staged_zeros)
        jax.block_until_ready(outs)
        return outs

    def results(outs):
        return [
            {
                nm: np.asarray(outs[i]).reshape(n_cores, *out_avals[i].shape)[c]
                for i, nm in enumerate(out_names)
            }
            for c in range(n_cores)
        ]

    return dict(run=run, stage=stage, zeros=zeros, results=results)


def _get_runner():
    if "runner" not in _CACHED:
        nc = build_kernel()
        _CACHED["nc"] = nc
        _CACHED["runner"] = make_runner(nc)
    return _CACHED["runner"]


def kernel(x, xpos, w_qkv, b_qkv, w_proj, b_proj):
    import hashlib

    r = _get_runner()
    h = hashlib.sha1()
    for a in (x, xpos, w_qkv, b_qkv, w_proj, b_proj):
        a = np.asarray(a)
        h.update(str(a.shape).encode())
        h.update(np.ascontiguousarray(a).tobytes())
    key = h.hexdigest()
    if _CACHED.get("staged_key") != key:
        in_maps = prep_in_maps(x, xpos, w_qkv, b_qkv, w_proj, b_proj)
        _CACHED["staged"] = r["stage"](in_maps)
        _CACHED["staged_key"] = key
    outs = r["run"](_CACHED["staged"], r["zeros"]())
    res = r["results"](outs)
    out = np.empty((B, N_FULL, C), np.float32)
    for core in range(8):
        b, q0 = core // 4, (core % 4) * NQ_FULL
        out[b, q0 : q0 + NQ_FULL] = res[core]["y"]
    return out


# revision 19
# speedup vs baseline: 1.0797x; 1.0797x over previous
"""Self-contained Trainium2 Bass kernel for a 2D-RoPE multi-head attention block.

Full module: qkv = x @ w_qkv.T + b_qkv; 2D rope on q,k; softmax attention;
out proj.  B=2, N=2048, C=768, H=12, Dh=64.

Sharding: 8 cores = 2 batches x 4 query-chunks of 512 tokens.  Each core
computes K,V for its batch's full 2048 tokens (replicated within the
4-core group) and Q for its own 512-token chunk; attention and the output
projection are then fully local.  Host only slices/transposes inputs and
concatenates outputs.
"""

import sys

import numpy as np

for _p in ("/opt/trn_rl_repo", "/root/.axon_site/_ro/trn_rl_repo"):
    if _p not in sys.path:
        sys.path.append(_p)

import concourse.bass as bass
import concourse.tile as tile
from concourse import mybir
from concourse.vector_clock import ScopedClock
import bass_rust

F32 = mybir.dt.float32
F32R = mybir.dt.float32r
BF16 = mybir.dt.bfloat16
AF = mybir.ActivationFunctionType

B, N_FULL, C, H, DH = 2, 2048, 768, 12, 64
NQ_FULL = N_FULL // 4  # 512 query rows per core
CT = C // 128  # contraction tiles
ROPE_FREQ = 100.0


class _TC(tile.TileContext):
    """TileContext whose tail drain splits sem waits one-per-instruction.

    The walrus in this container rejects >1 sync wait on a CTRL-class
    instruction ("Too many sync wait commands"), and the stock tail drain
    carries one wait per active processor.
    """

    def _drain_and_barrier(self, tick_clock, wait_clock):
        nc = self.nc
        drain_inst = nc.sync.drain()
        wait_clock.add_sem_waits(
            drain_inst.ins, ScopedClock({None: tick_clock.global_clock})
        )
        si = drain_inst.ins.sync_info
        waits = list(si.on_wait) if si is not None else []
        if len(waits) > 1:
            drain_inst.ins.sync_info = bass_rust.SyncInfo(
                on_wait=waits[:1], on_update=si.on_update
            )
            for w in waits[1:]:
                n = nc.sync.nop(nofuse=True)
                n.ins.sync_info = bass_rust.SyncInfo(on_wait=[w], on_update=[])
        nc.all_engine_barrier()
        popped = nc._tile_sem_poison_stack.pop()
        assert popped is self._sem_poison
        nc.clear_and_free_semaphores(list(self.sems.allocated().values()))
        nc.all_engine_barrier()


def _split_multi_waits(nc):
    """This container's walrus accepts at most ONE sync wait per instruction.

    Move extra waits onto fresh NOPs inserted just before the instruction
    (same engine, same basic block) - semantically identical stalls.
    """
    for f in nc.m.functions:
        for bb in f.blocks:
            il = bb.instructions
            if not any(
                inst.sync_info is not None and len(inst.sync_info.on_wait) > 1
                for inst in il
            ):
                continue
            new = []
            for inst in il:
                si = inst.sync_info
                if si is not None and len(si.on_wait) > 1:
                    waits = list(si.on_wait)
                    for w in waits[:-1]:
                        nop = mybir.InstNoOp(
                            name=nc.get_next_instruction_name(), ins=[], outs=[]
                        )
                        nop.engine = inst.engine
                        nop.sync_info = bass_rust.SyncInfo(
                            on_wait=[w], on_update=[]
                        )
                        nc.register_instruction(nop)
                        new.append(nop)
                    inst.sync_info = bass_rust.SyncInfo(
                        on_wait=[waits[-1]], on_update=si.on_update
                    )
                new.append(inst)
            il[:] = new


def build_kernel(n=N_FULL, nq=NQ_FULL, repeat=1):
    """Build the per-core SPMD program.  n = kv tokens, nq = query rows.

    repeat>1 re-emits the whole body N times (timing calibration only).
    """
    assert n % 512 == 0 or n in (128, 256, 512)
    jt = n // 128       # key tiles
    jg2 = jt // 2       # key-tile pairs (psum/exp groups)
    vtt = n // 128      # v token tiles
    hp_n = H // 2       # head pairs

    nc = bass.Bass("TRN2")
    xT = nc.dram_tensor("xT", [C, n], F32, kind="ExternalInput")
    xqT = nc.dram_tensor("xqT", [C, nq], F32, kind="ExternalInput")
    cosT = nc.dram_tensor("cosT", [128, n], F32, kind="ExternalInput")
    sinT = nc.dram_tensor("sinT", [128, n], F32, kind="ExternalInput")
    cosq = nc.dram_tensor("cosq", [128, nq], F32, kind="ExternalInput")
    sinq = nc.dram_tensor("sinq", [128, nq], F32, kind="ExternalInput")
    wkqT = nc.dram_tensor("wkqT", [C, 2 * C], F32, kind="ExternalInput")
    wvT = nc.dram_tensor("wvT", [C, C], F32, kind="ExternalInput")
    wpT = nc.dram_tensor("wpT", [C, C], F32, kind="ExternalInput")
    bkq = nc.dram_tensor("bkq", [128, 2 * CT], F32, kind="ExternalInput")
    bv = nc.dram_tensor("bv", [1, C], F32, kind="ExternalInput")
    bp = nc.dram_tensor("bp", [1, C], F32, kind="ExternalInput")
    ones = nc.dram_tensor("ones", [1, 512], F32, kind="ExternalInput")
    r2t = nc.dram_tensor("r2t", [128, 128], F32, kind="ExternalInput")
    y = nc.dram_tensor("y", [nq, C], F32, kind="ExternalOutput")

    with _TC(nc) as tc:
      for _rep in range(repeat):
        with (
            tc.tile_pool(name="const", bufs=1) as pc,
            tc.tile_pool(name="kq", bufs=1) as pkq,
            tc.tile_pool(name="oT", bufs=1) as po,
            tc.tile_pool(name="v", bufs=1) as pv,
            tc.tile_pool(name="x", bufs=1) as px,
        ):
            # ---- constants ----
            bkq_sb = pc.tile([128, 2 * CT], F32, tag="bkq", name="bkq")
            nc.sync.dma_start(bkq_sb[:], bkq[:])
            bv_sb = pc.tile([1, C], F32, tag="bv", name="bv")
            nc.sync.dma_start(bv_sb[:], bv[:])
            bp_sb = pc.tile([1, C], F32, tag="bp", name="bp")
            nc.sync.dma_start(bp_sb[:], bp[:])
            ones_sb = pc.tile([1, 512], F32, tag="ones", name="ones")
            nc.sync.dma_start(ones_sb[:], ones[:])
            r2t_sb = pc.tile([128, 128], F32, tag="r2t", name="r2t")
            nc.sync.dma_start(r2t_sb[:], r2t[:])
            ones64_sb = pc.tile([65, 64], F32, tag="ones64", name="ones64")
            nc.vector.memset(ones64_sb[64:65, :], 1.0)
            cos_sb = pc.tile([128, n], F32, tag="cos", name="cos")
            nc.sync.dma_start(cos_sb[:], cosT[:])
            sin_sb = pc.tile([128, n], F32, tag="sin", name="sin")
            nc.sync.dma_start(sin_sb[:], sinT[:])
            cq_sb = pc.tile([128, nq], F32, tag="cosq", name="cosq")
            nc.sync.dma_start(cq_sb[:], cosq[:])
            sq_sb = pc.tile([128, nq], F32, tag="sinq", name="sinq")
            nc.sync.dma_start(sq_sb[:], sinq[:])

            # ---- resident tensors ----
            kT = []  # roped K^T, bf16, head-pair-major: tile p rows = heads 2p,2p+1
            for p in range(hp_n):
                kT.append(pkq.tile([128, n], BF16, tag=f"kT{p}", name=f"kT{p}"))
            qT = []
            for p in range(hp_n):
                qT.append(pkq.tile([128, nq], BF16, tag=f"qT{p}", name=f"qT{p}"))
            oT = []  # O^T feature-major (post-normalize), f32
            for p in range(hp_n):
                oT.append(po.tile([128, nq], F32, tag=f"oT{p}", name=f"oT{p}"))
            v_sb = []  # token-major V with ones col per head: [128, H*65] bf16
            for t in range(vtt):
                v_sb.append(pv.tile([128, H * 65], BF16, tag=f"v{t}", name=f"v{t}"))
            x_sb = []
            for c in range(CT):
                x_sb.append(px.tile([128, n], F32, tag=f"x{c}", name=f"x{c}"))
                nc.sync.dma_start(x_sb[c][:], xT[c * 128 : (c + 1) * 128, :])
            xq_sb = []
            for c in range(CT):
                xq_sb.append(px.tile([128, nq], F32, tag=f"xq{c}", name=f"xq{c}"))
                nc.sync.dma_start(xq_sb[c][:], xqT[c * 128 : (c + 1) * 128, :])

            # ======== phase 1a: K^T and Q^T (feature-major) + rope ========
            with (
                tc.tile_pool(name="w1", bufs=1) as pw1,
                tc.tile_pool(name="tmp", bufs=2) as ptmp,
                tc.tile_pool(name="psK", bufs=2, space="PSUM") as psK,
                tc.tile_pool(name="psR", bufs=2, space="PSUM") as psR,
            ):
                wkq_sb = []
                for c in range(CT):
                    wkq_sb.append(pw1.tile([128, 2 * C], F32, tag=f"wkq{c}", name=f"wkq{c}"))
                    nc.sync.dma_start(
                        wkq_sb[c][:], wkqT[c * 128 : (c + 1) * 128, :]
                    )

                def rope_block(dst, ps_pool, w_off, o, bias_col, src, src_w,
                               cos_s, sin_s):
                    # dst[o][:, :] (bf16) = rope(W[:, w_off+o*128] ^T @ src + b)
                    nchunks = max(1, src_w // 1024)
                    chw = min(1024, src_w)
                    for ch in range(nchunks):
                        ps = ps_pool.tile([128, chw], F32, tag="k", name="k")
                        nmm = max(1, chw // 512)
                        mmw = min(512, chw)
                        for nn_ in range(nmm):
                            col0 = ch * chw + nn_ * mmw
                            for c in range(CT):
                                nc.tensor.matmul(
                                    ps[:, nn_ * mmw : (nn_ + 1) * mmw],
                                    wkq_sb[c][:, w_off + o * 128 : w_off + (o + 1) * 128].bitcast(F32R),
                                    src[c][:, col0 : col0 + mmw].bitcast(F32R),
                                    start=(c == 0),
                                    stop=(c == CT - 1),
                                )
                        # evac + bias (per-partition scalar add), keep f32
                        tev = ptmp.tile([128, chw], F32, tag="tev", name="tev")
                        nc.vector.tensor_scalar_add(
                            tev[:], ps[:], bkq_sb[:, bias_col : bias_col + 1]
                        )
                        # rotate via PE: psR = R2 @ tev
                        pr = psR.tile([128, chw], F32, tag="r", name="r")
                        for nn_ in range(nmm):
                            nc.tensor.matmul(
                                pr[:, nn_ * mmw : (nn_ + 1) * mmw],
                                r2t_sb[:].bitcast(F32R),
                                tev[:, nn_ * mmw : (nn_ + 1) * mmw].bitcast(F32R),
                                start=True,
                                stop=True,
                            )
                        # combine: dst = tev*cos + (R2 tev)*sin
                        c0 = ch * chw
                        t1 = ptmp.tile([128, chw], F32, tag="t1", name="t1")
                        nc.gpsimd.tensor_mul(
                            t1[:], tev[:], cos_s[:, c0 : c0 + chw]
                        )
                        t2 = ptmp.tile([128, chw], F32, tag="t2", name="t2")
                        nc.vector.tensor_mul(
                            t2[:], pr[:], sin_s[:, c0 : c0 + chw]
                        )
                        nc.vector.tensor_add(
                            dst[:, c0 : c0 + chw], t1[:], t2[:]
                        )

                for o in range(CT):  # K head-pair tiles
                    rope_block(kT[o], psK, 0, o, o, x_sb, n, cos_sb, sin_sb)
                for o in range(CT):  # Q head-pair tiles
                    rope_block(qT[o], psK, C, o, CT + o, xq_sb, nq, cq_sb, sq_sb)

            # ======== phase 1b: V (token-major) ========
            with (
                tc.tile_pool(name="w2", bufs=1) as pw2,
                tc.tile_pool(name="psV", bufs=2, space="PSUM") as psV,
            ):
                wv_sb = []
                for c in range(CT):
                    wv_sb.append(pw2.tile([128, C], F32, tag=f"wv{c}", name=f"wv{c}"))
                    nc.sync.dma_start(wv_sb[c][:], wvT[c * 128 : (c + 1) * 128, :])
                for t in range(vtt):
                    ps = psV.tile([128, C], F32, tag="v", name="v")
                    for e0, ew in ((0, 512), (512, 256)):
                        for c in range(CT):
                            nc.tensor.matmul(
                                ps[:, e0 : e0 + ew],
                                x_sb[c][:, t * 128 : (t + 1) * 128].bitcast(F32R),
                                wv_sb[c][:, e0 : e0 + ew].bitcast(F32R),
                                start=(c == 0),
                                stop=False,
                            )
                        nc.tensor.matmul(  # + ones[t] x bv  (bias)
                            ps[:, e0 : e0 + ew],
                            ones_sb[0:1, 0:128].bitcast(F32R),
                            bv_sb[0:1, e0 : e0 + ew].bitcast(F32R),
                            start=False,
                            stop=True,
                        )
                    vview = v_sb[t][:].rearrange("p (h e) -> p h e", e=65)
                    psview = ps[:].rearrange("p (h d) -> p h d", d=DH)
                    nc.vector.tensor_copy(vview[:, :, 0:DH], psview[:, :, :])
                    nc.vector.memset(vview[:, :, DH : DH + 1], 1.0)

            # ======== phase 2: attention (S^T layout, flash-free) ========
            with (
                tc.tile_pool(name="pT", bufs=2) as ppt,
                tc.tile_pool(name="inv", bufs=2) as pinv,
                tc.tile_pool(name="oscr", bufs=2) as poscr,
                tc.tile_pool(name="psS", bufs=2, space="PSUM") as psS,
                tc.tile_pool(name="psO", bufs=2, space="PSUM") as psO,
            ):
                for hp in range(hp_n):
                    hA, hB = 2 * hp, 2 * hp + 1
                    oA = psO.tile([65, nq], F32, tag="oA", name="oA")
                    oB = psO.tile([65, nq], F32, tag="oB", name="oB")
                    for g in range(jg2):
                        sA = psS.tile([128, 2 * nq], F32, tag="s", name="s")
                        sB = psS.tile([128, 2 * nq], F32, tag="s", name="s")
                        for jj in range(2):
                            j = 2 * g + jj
                            nc.tensor.matmul(
                                sA[:, jj * nq : (jj + 1) * nq],
                                kT[hp][0:64, j * 128 : (j + 1) * 128],
                                qT[hp][0:64, :],
                                start=True, stop=True,
                                tile_position=(0, 0),
                            )
                            nc.tensor.matmul(
                                sB[:, jj * nq : (jj + 1) * nq],
                                kT[hp][64:128, j * 128 : (j + 1) * 128],
                                qT[hp][64:128, :],
                                start=True, stop=True,
                                tile_position=(64, 0),
                            )
                        pA = ppt.tile([128, 2 * nq], BF16, tag="pA", name="pA")
                        nc.scalar.activation(pA[:], sA[:], AF.Exp, scale=0.125)
                        pB = ppt.tile([128, 2 * nq], BF16, tag="pB", name="pB")
                        nc.scalar.activation(pB[:], sB[:], AF.Exp, scale=0.125)
                        for jj in range(2):
                            j = 2 * g + jj
                            nc.tensor.matmul(
                                oA[:, :],
                                v_sb[j][:, hA * 65 : (hA + 1) * 65],
                                pA[:, jj * nq : (jj + 1) * nq],
                                start=(j == 0), stop=(j == jt - 1),
                            )
                            nc.tensor.matmul(
                                oB[:, :],
                                v_sb[j][:, hB * 65 : (hB + 1) * 65],
                                pB[:, jj * nq : (jj + 1) * nq],
                                start=(j == 0), stop=(j == jt - 1),
                            )
                    # normalize: O = O / denom (denom = row 64).
                    # reciprocal at partition 64, broadcast to partitions
                    # 0-63 via a K=1 outer product with a ones row.
                    for which, ops in (("A", oA), ("B", oB)):
                        invr = pinv.tile([65, nq], F32, tag=f"invr{which}",
                                         name=f"invr{which}")
                        with nc.allow_low_precision(reason="feeds f32r matmul"):
                            nc.vector.reciprocal(invr[64:65, :], ops[64:65, :])
                        pib = psS.tile([64, nq], F32, tag="s", name="pib")
                        nc.tensor.matmul(
                            pib[:, :],
                            ones64_sb[64:65, :].bitcast(F32R),
                            invr[64:65, :].bitcast(F32R),
                            start=True, stop=True,
                        )
                        invs = pinv.tile([64, nq], F32, tag=f"invs{which}",
                                         name=f"invs{which}")
                        nc.vector.tensor_copy(invs[:, :], pib[:, :])
                        if which == "A":
                            nc.vector.tensor_mul(
                                oT[hp][0:64, :], ops[0:64, :], invs[:, :]
                            )
                        else:
                            osc = poscr.tile([64, nq], F32, tag="oscr",
                                             name="oscr")
                            nc.vector.tensor_mul(
                                osc[:, :], ops[0:64, :], invs[:, :]
                            )
                            # partition shift 0-63 -> 64-127 via DMA
                            nc.sync.dma_start(oT[hp][64:128, :], osc[:, :])

            # ======== phase 3: output projection ========
            with (
                tc.tile_pool(name="w3", bufs=1) as pw3,
                tc.tile_pool(name="psY", bufs=2, space="PSUM") as psY,
            ):
                wp_sb = []
                for c in range(CT):
                    wp_sb.append(pw3.tile([128, C], F32, tag=f"wp{c}", name=f"wp{c}"))
                    nc.sync.dma_start(wp_sb[c][:], wpT[c * 128 : (c + 1) * 128, :])
                for t in range(nq // 128):
                    ps = psY.tile([128, C], F32, tag="y", name="y")
                    for e0, ew in ((0, 512), (512, 256)):
                        for f in range(CT):
                            nc.tensor.matmul(
                                ps[:, e0 : e0 + ew],
                                oT[f][:, t * 128 : (t + 1) * 128].bitcast(F32R),
                                wp_sb[f][:, e0 : e0 + ew].bitcast(F32R),
                                start=(f == 0),
                                stop=False,
                            )
                        nc.tensor.matmul(  # + ones[t] x bp  (bias)
                            ps[:, e0 : e0 + ew],
                            ones_sb[0:1, 0:128].bitcast(F32R),
                            bp_sb[0:1, e0 : e0 + ew].bitcast(F32R),
                            start=False,
                            stop=True,
                        )
                    ysb = pw3.tile([128, C], F32, tag="ysb", name="ysb", bufs=2)
                    nc.scalar.copy(ysb[:], ps[:])
                    nc.sync.dma_start(y[t * 128 : (t + 1) * 128, :], ysb[:])

    _split_multi_waits(nc)
    return nc


# ---------------------------------------------------------------------------
# host-side prep
# ---------------------------------------------------------------------------

def _rope_tables(xpos_b):
    """cos/sin tables [128, N] (64 dims stacked twice for head-pair tiles)."""
    d = 32
    inv = (
        1.0
        / (np.float32(ROPE_FREQ) ** (np.arange(0, d, 2, dtype=np.float32) / np.float32(d)))
    ).astype(np.float32)
    py = xpos_b[:, 0].astype(np.float32)[:, None] * inv  # [N, 16]
    px = xpos_b[:, 1].astype(np.float32)[:, None] * inv
    emb = np.concatenate([py, py, px, px], axis=1)  # [N, 64]
    cos = np.cos(emb).T.astype(np.float32)  # [64, N]
    sin = np.sin(emb).T.astype(np.float32)
    return (
        np.ascontiguousarray(np.concatenate([cos, cos], axis=0)),
        np.ascontiguousarray(np.concatenate([sin, sin], axis=0)),
    )


def _r2t():
    r32 = np.zeros((32, 32), np.float32)
    r32[0:16, 16:32] = -np.eye(16, dtype=np.float32)
    r32[16:32, 0:16] = np.eye(16, dtype=np.float32)
    r = np.zeros((128, 128), np.float32)
    for i in range(4):
        r[i * 32 : (i + 1) * 32, i * 32 : (i + 1) * 32] = r32
    return np.ascontiguousarray(r.T)


def prep_in_maps(x, xpos, w_qkv, b_qkv, w_proj, b_proj, n=N_FULL, nq=NQ_FULL,
                 n_cores=8):
    x = np.asarray(x, dtype=np.float32)
    xpos = np.asarray(xpos)
    w_qkv = np.asarray(w_qkv, dtype=np.float32)
    b_qkv = np.asarray(b_qkv, dtype=np.float32)
    w_proj = np.asarray(w_proj, dtype=np.float32)
    b_proj = np.asarray(b_proj, dtype=np.float32)

    wkqT = np.ascontiguousarray(
        np.concatenate([w_qkv[C : 2 * C], w_qkv[0:C]], axis=0).T
    )  # [C, 2C]: K cols then Q cols
    wvT = np.ascontiguousarray(w_qkv[2 * C : 3 * C].T)
    wpT = np.ascontiguousarray(w_proj.T)
    bkq = np.ascontiguousarray(
        np.concatenate([b_qkv[C : 2 * C], b_qkv[0:C]]).reshape(2 * CT, 128).T
    )  # [128, 12]
    bv = np.ascontiguousarray(b_qkv[2 * C : 3 * C][None])
    bp = np.ascontiguousarray(b_proj[None])
    ones = np.ones((1, 512), np.float32)
    r2t = _r2t()

    shared = dict(wkqT=wkqT, wvT=wvT, wpT=wpT, bkq=bkq, bv=bv, bp=bp,
                  ones=ones, r2t=r2t)

    per_batch = []
    for b in range(x.shape[0]):
        xT = np.ascontiguousarray(x[b, :n].T)
        cosT, sinT = _rope_tables(np.asarray(xpos[b, :n]))
        per_batch.append((xT, cosT, sinT))

    chunks_per_batch = max(1, n_cores // x.shape[0])
    in_maps = []
    for core in range(n_cores):
        b = core // chunks_per_batch
        q0 = (core % chunks_per_batch) * nq
        xT, cosT, sinT = per_batch[b]
        in_maps.append(
            dict(
                xT=xT,
                xqT=np.ascontiguousarray(xT[:, q0 : q0 + nq]),
                cosT=cosT,
                sinT=sinT,
                cosq=np.ascontiguousarray(cosT[:, q0 : q0 + nq]),
                sinq=np.ascontiguousarray(sinT[:, q0 : q0 + nq]),
                **shared,
            )
        )
    return in_maps


_CACHED = {}


def make_runner(nc, n_cores=8):
    """One cached jitted shard_map callable for the whole session.

    Mirrors bass2jax.run_bass_via_pjrt but hoists jit construction so
    repeat calls skip retracing, and exposes device staging for timing.
    """
    import jax
    from jax.experimental.shard_map import shard_map
    from jax.sharding import Mesh, PartitionSpec

    from concourse import bass2jax, mybir as mb

    bass2jax.install_neuronx_cc_hook()
    assert nc.dbg_addr is None

    partition_name = nc.partition_id_tensor.name if nc.partition_id_tensor else None
    in_names, out_names, out_avals = [], [], []
    for alloc in nc.m.functions[0].allocations:
        if not isinstance(alloc, mb.MemoryLocationSet):
            continue
        name = alloc.memorylocations[0].name
        if alloc.kind == "ExternalInput":
            if name != partition_name:
                in_names.append(name)
        elif alloc.kind == "ExternalOutput":
            out_names.append(name)
            out_avals.append(
                jax.core.ShapedArray(tuple(alloc.tensor_shape), mb.dt.np(alloc.dtype))
            )
    n_params = len(in_names)
    all_in = list(in_names) + list(out_names)
    if partition_name is not None:
        all_in.append(partition_name)
    donate = tuple(range(n_params, n_params + len(out_names)))

    def _body(*args):
        operands = list(args)
        if partition_name is not None:
            operands.append(bass2jax.partition_id_tensor())
        outs = bass2jax._bass_exec_p.bind(
            *operands,
            out_avals=tuple(out_avals),
            in_names=tuple(all_in),
            out_names=tuple(out_names),
            lowering_input_output_aliases=(),
            sim_require_finite=True,
            sim_require_nnan=True,
            nc=nc,
        )
        return tuple(outs)

    devices = jax.devices()[:n_cores]
    mesh = Mesh(np.asarray(devices), ("core",))
    n_outs = len(out_names)
    sharded = jax.jit(
        shard_map(
            _body,
            mesh=mesh,
            in_specs=(PartitionSpec("core"),) * (n_params + n_outs),
            out_specs=(PartitionSpec("core"),) * n_outs,
            check_rep=False,
        ),
        donate_argnums=donate,
        keep_unused=True,
    )

    def stage(in_maps):
        """host in_maps -> concatenated device arrays (inputs only)."""
        concat = [
            jax.device_put(
                np.concatenate([np.asarray(in_maps[c][nm]) for c in range(n_cores)], axis=0)
            )
            for nm in in_names
        ]
        jax.block_until_ready(concat)
        return concat

    def zeros():
        zs = [
            jax.device_put(
                np.zeros((n_cores * av.shape[0], *av.shape[1:]), av.dtype)
            )
            for av in out_avals
        ]
        jax.block_until_ready(zs)
        return zs

    def run(staged_in, staged_zeros):
        outs = sharded(*staged_in, *staged_zeros)
        jax.block_until_ready(outs)
        return outs

    def results(outs):
        return [
            {
                nm: np.asarray(outs[i]).reshape(n_cores, *out_avals[i].shape)[c]
                for i, nm in enumerate(out_names)
            }
            for c in range(n_cores)
        ]

    return dict(run=run, stage=stage, zeros=zeros, results=results)


def _get_runner():
    if "runner" not in _CACHED:
        nc = build_kernel()
        _CACHED["nc"] = nc
        _CACHED["runner"] = make_runner(nc)
    return _CACHED["runner"]


def kernel(x, xpos, w_qkv, b_qkv, w_proj, b_proj):
    import hashlib

    r = _get_runner()
    h = hashlib.sha1()
    for a in (x, xpos, w_qkv, b_qkv, w_proj, b_proj):
        a = np.asarray(a)
        h.update(str(a.shape).encode())
        h.update(np.ascontiguousarray(a).tobytes())
    key = h.hexdigest()
    if _CACHED.get("staged_key") != key:
        in_maps = prep_in_maps(x, xpos, w_qkv, b_qkv, w_proj, b_proj)
        _CACHED["staged"] = r["stage"](in_maps)
        _CACHED["staged_key"] = key
    outs = r["run"](_CACHED["staged"], r["zeros"]())
    res = r["results"](outs)
    out = np.empty((B, N_FULL, C), np.float32)
    for core in range(8):
        b, q0 = core // 4, (core % 4) * NQ_FULL
        out[b, q0 : q0 + NQ_FULL] = res[core]["y"]
    return out
